# revision 1
# baseline (speedup 1.0000x reference)
"""Trainium2 Bass kernel for batched 64-point DCT (flattened-patch GEMM).

Reference computation: out = x.reshape(b, -1, 64) @ K, reshaped back.
Pure data parallel over 8 NeuronCores: core i handles batch i as a
[49152, 64] x [64, 64] GEMM.

Sharding strategy: while slicing the batch on the host, each core's input
is also laid out as xth[(z, s), pair] = x[2*pair + z, s] -- a [128, 24576]
matrix whose partition dim is (patch-parity, DCT-coefficient). With that
layout the device kernel needs no on-chip transposes at all:

  1. DMA in [128, 2048] tiles (8 KB contiguous per partition).
  2. One fp32 matmul per 128 pair-columns: stationary = data slice
     [128, 128], moving = blockdiag(K, K) [128, 128]:
       out[q, z*64+f] = sum_s x[2*(base+q)+z, s] * K[s, f]
     which is exactly two consecutive patches' outputs per partition --
     the natural DRAM layout of y.
  3. Four matmuls share one PSUM bank [128, 512]; a single DVE/ACT copy
     moves them to SBUF staging. The host pre-permutes pair columns so
     each output partition holds 16 consecutive pairs, making the store
     a contiguous 8 KB per partition as well.
"""

import numpy as np

import concourse.mybir as mybir
from concourse import bacc
from concourse.bass_utils import run_bass_kernel_spmd
from concourse.tile import TileContext

P = 128   # SBUF partitions
S = 64    # DCT size (contraction dim)
M = 16    # matmuls per macro-tile (128 pairs each)
N_CORES = 8
PAIRS_PER_TILE = P * M          # 2048 pair-columns per macro-tile
PATCHES_PER_TILE = 2 * PAIRS_PER_TILE


def build_kernel(n_patches: int):
    assert n_patches % PATCHES_PER_TILE == 0
    n_tiles = n_patches // PATCHES_PER_TILE
    n_pairs = n_patches // 2
    nc = bacc.Bacc(
        "TRN2",
        target_bir_lowering=False,
        debug=False,
        enable_asserts=False,
        num_devices=N_CORES,
    )
    # xth[(z*64+s), pair] = x[2*pair+z, s], prepared host-side.
    x = nc.dram_tensor("x", [P, n_pairs], mybir.dt.float32, kind="ExternalInput")
    # host-prepared blockdiag(K, K)
    k = nc.dram_tensor("k", [P, P], mybir.dt.float32, kind="ExternalInput")
    y = nc.dram_tensor("y", [n_patches, S], mybir.dt.float32, kind="ExternalOutput")

    xv = x.ap().rearrange("r (t n) -> t r n", n=PAIRS_PER_TILE)
    # device column c = t*2048 + m*128 + q maps to pair p = t*2048 + q*16 + m
    # (host pre-permutes), so out partition q accumulates 16 consecutive
    # pairs -> patch = t*4096 + q*32 + 2m + z and the store is a contiguous
    # 8KB per partition.
    yv = y.ap().rearrange("(t q m z) s -> t q m (z s)", q=P, m=M, z=2)

    with TileContext(nc) as tc:
        with (
            tc.tile_pool(name="consts", bufs=1) as consts,
            tc.tile_pool(name="xin", bufs=6) as x_pool,
            tc.tile_pool(name="outsb", bufs=6) as out_pool,
            tc.tile_pool(name="pout", bufs=8, space="PSUM") as pout_pool,
        ):
            kblk = consts.tile([P, P], mybir.dt.float32)
            first_x = x_pool.tile(
                [P, PAIRS_PER_TILE], mybir.dt.float32, tag="x_tile",
                name="x_head",
            )
            # tile-0 load is emitted first so it heads the Sync queue;
            # kblk rides the otherwise-idle Scalar queue.
            nc.sync.dma_start(out=first_x[:], in_=xv[0])
            nc.scalar.dma_start(out=kblk[:], in_=k.ap())

            for ti in range(n_tiles):
                if ti == 0:
                    x_tile = first_x
                else:
                    x_tile = x_pool.tile(
                        [P, PAIRS_PER_TILE], mybir.dt.float32, tag="x_tile",
                        name=f"x_body{ti}",
                    )
                    nc.sync.dma_start(out=x_tile[:], in_=xv[ti])
                out_sb = out_pool.tile([P, M, P], mybir.dt.float32)
                for g in range(M // 4):
                    po = pout_pool.tile([P, 4, P], mybir.dt.float32)
                    for mm in range(4):
                        m = 4 * g + mm
                        nc.tensor.matmul(
                            po[:, mm, :],
                            lhsT=x_tile[:, P * m : P * (m + 1)],
                            rhs=kblk[:],
                            start=True,
                            stop=True,
                        )
                    if g % 2 == 0:
                        nc.vector.tensor_copy(
                            out_sb[:, 4 * g : 4 * (g + 1), :], po[:]
                        )
                    else:
                        nc.scalar.copy(out_sb[:, 4 * g : 4 * (g + 1), :], po[:])
                # store on the Scalar hwdge queue so it overlaps the
                # Sync-queue input stream
                nc.scalar.dma_start(out=yv[ti], in_=out_sb[:])
    nc.compile()
    return nc


def shard_input(x_core: np.ndarray) -> np.ndarray:
    """[n_patches, 64] -> [128, n_pairs] device layout.

    Device column c = t*2048 + m*128 + q holds pair p = t*2048 + q*16 + m
    (patch = 2p + z), with row r = z*64 + s.
    """
    n = x_core.shape[0]
    t = n // PATCHES_PER_TILE
    x5 = x_core.reshape(t, P, M, 2, S)          # [t, q, m, z, s]
    return np.ascontiguousarray(
        x5.transpose(3, 4, 0, 2, 1).reshape(P, n // 2)  # [(z s), (t m q)]
    )


def kernel(inputs, kernel):
    x_full = np.ascontiguousarray(np.asarray(inputs, dtype=np.float32))
    kmat = np.ascontiguousarray(np.asarray(kernel, dtype=np.float32))
    b, c, h, w = x_full.shape
    assert b == N_CORES, f"expected batch {N_CORES}, got {b}"
    n_patches = c * h * w // S
    nc = build_kernel(n_patches)
    kblk_host = np.zeros((P, P), dtype=np.float32)
    kblk_host[:S, :S] = kmat
    kblk_host[S:, S:] = kmat
    in_maps = [
        {"x": shard_input(x_full[i].reshape(n_patches, S)), "k": kblk_host}
        for i in range(b)
    ]
    res = run_bass_kernel_spmd(nc, in_maps, core_ids=list(range(N_CORES)))
    out = np.stack(
        [res.results[i]["y"].reshape(c, h, w) for i in range(b)], axis=0
    )
    return out



# revision 3
# speedup vs baseline: 1.6788x; 1.6788x over previous
"""Trainium2 Bass kernel for batched 64-point DCT (flattened-patch GEMM).

Reference computation: out = x.reshape(b, -1, 64) @ K, reshaped back.
Pure data parallel over 8 NeuronCores: core i handles batch i as a
[49152, 64] x [64, 64] GEMM.

The rel-err gate is 2e-2; fp16 end-to-end lands at ~4e-4, so all HBM
traffic moves as fp16 -- half the bytes of fp32 on a memory-bound
kernel. The host casts fp32->fp16 while packing and upcasts the fp16
results; the device matmuls fp16 with fp32 PSUM accumulate and casts
back to fp16 on the PSUM->SBUF drain.

Device layout: both input and output live as [128, n_pairs] with
partition r = z*64 + s (patch-parity, coefficient) and free dim = pair
index p (patch = 2p + z):

    xth[z*64+s, p] = x[2p+z, s]        yth[z*64+f, p] = y[2p+z, f]

With stationary = blockdiag(K, K) (loaded into the PE array once for
the whole kernel) each matmul streams 512 pair-columns of x straight
into one PSUM bank:

    po[z*64+f, q] = sum_s K[s, f] * x[2q+z, s]

so the output needs no on-chip transpose -- it is stored in the same
transposed layout and the host un-transposes. Per 1 MB macro-tile
(4096 pairs): one load, 8 matmuls, 4 PSUM->SBUF cast-copies
(alternating DVE/ACT), one store. Loads ride the Sync HWDGE ring;
stores alternate between the Scalar HWDGE ring and the GpSimd SWDGE
ring so consecutive stores' HBM write-receipt stalls overlap.
"""

import numpy as np

import concourse.mybir as mybir
from concourse import bacc
from concourse.bass_utils import run_bass_kernel_spmd
from concourse.tile import TileContext

P = 128    # SBUF partitions
S = 64     # DCT size (contraction dim)
MM = 512   # moving columns per matmul (one PSUM bank of fp32)
N_CORES = 8
PAIRS_PER_TILE = 4096           # 1 MB fp16 per DMA tile
MATMULS_PER_TILE = PAIRS_PER_TILE // MM   # 8
DT = mybir.dt.float16
NPDT = np.float16


def build_kernel(n_patches: int):
    assert n_patches % (2 * PAIRS_PER_TILE) == 0
    n_pairs = n_patches // 2
    n_tiles = n_pairs // PAIRS_PER_TILE
    nc = bacc.Bacc(
        "TRN2",
        target_bir_lowering=False,
        debug=False,
        enable_asserts=False,
        num_devices=N_CORES,
    )
    x = nc.dram_tensor("x", [P, n_pairs], DT, kind="ExternalInput")
    k = nc.dram_tensor("k", [P, P], DT, kind="ExternalInput")
    y = nc.dram_tensor("y", [P, n_pairs], DT, kind="ExternalOutput")

    xv = x.ap().rearrange("r (t n) -> t r n", n=PAIRS_PER_TILE)
    yv = y.ap().rearrange("r (t n) -> t r n", n=PAIRS_PER_TILE)

    with TileContext(nc) as tc:
        with (
            tc.tile_pool(name="consts", bufs=1) as consts,
            tc.tile_pool(name="xin", bufs=3) as x_pool,
            tc.tile_pool(name="outsb", bufs=3) as out_pool,
            tc.tile_pool(name="pout", bufs=4, space="PSUM") as pout_pool,
        ):
            kblk = consts.tile([P, P], DT)
            first_x = x_pool.tile(
                [P, PAIRS_PER_TILE], DT, tag="x_tile", name="x_head"
            )
            # tile-0 load heads the Sync queue; kblk rides the
            # otherwise-idle Scalar queue.
            nc.sync.dma_start(out=first_x[:], in_=xv[0])
            nc.scalar.dma_start(out=kblk[:], in_=k.ap())

            for ti in range(n_tiles):
                if ti == 0:
                    x_tile = first_x
                else:
                    x_tile = x_pool.tile(
                        [P, PAIRS_PER_TILE], DT, tag="x_tile",
                        name=f"x_body{ti}",
                    )
                    nc.sync.dma_start(out=x_tile[:], in_=xv[ti])
                out_sb = out_pool.tile([P, PAIRS_PER_TILE], DT)
                for g in range(MATMULS_PER_TILE // 2):
                    po = pout_pool.tile([P, 2 * MM], mybir.dt.float32)
                    for half in range(2):
                        c0 = (2 * g + half) * MM
                        nc.tensor.matmul(
                            po[:, half * MM : (half + 1) * MM],
                            lhsT=kblk[:],
                            rhs=x_tile[:, c0 : c0 + MM],
                            start=True,
                            stop=True,
                        )
                    dst = out_sb[:, 2 * g * MM : 2 * (g + 1) * MM]
                    if g % 2 == 0:
                        nc.vector.tensor_copy(dst, po[:])
                    else:
                        nc.scalar.copy(dst, po[:])
                # stores alternate Scalar HWDGE / GpSimd SWDGE so their
                # HBM write-receipt stalls overlap
                if ti % 2 == 0:
                    nc.scalar.dma_start(out=yv[ti], in_=out_sb[:])
                else:
                    nc.gpsimd.dma_start(out=yv[ti], in_=out_sb[:])
    nc.compile()
    return nc


def pack_input(x_core: np.ndarray) -> np.ndarray:
    """[n_patches, 64] fp32 -> [128, n_pairs] fp16 device layout."""
    x3 = x_core.reshape(-1, 2, S)                     # [pair, z, s]
    return np.ascontiguousarray(
        x3.transpose(1, 2, 0).reshape(P, -1).astype(NPDT)
    )


def unpack_output(y_dev: np.ndarray, n_patches: int) -> np.ndarray:
    """[128, n_pairs] fp16 device layout -> [n_patches, 64] fp32."""
    y3 = np.asarray(y_dev, dtype=np.float32).reshape(2, S, n_patches // 2)
    return y3.transpose(2, 0, 1).reshape(n_patches, S)


def make_in_maps(x_full: np.ndarray, kmat: np.ndarray) -> list[dict]:
    b = x_full.shape[0]
    n_patches = x_full[0].size // S
    kblk_host = np.zeros((P, P), dtype=NPDT)
    kblk_host[:S, :S] = kmat.astype(NPDT)
    kblk_host[S:, S:] = kmat.astype(NPDT)
    return [
        {"x": pack_input(x_full[i].reshape(n_patches, S)), "k": kblk_host}
        for i in range(b)
    ]


def kernel(inputs, kernel):
    x_full = np.asarray(inputs, dtype=np.float32)
    kmat = np.asarray(kernel, dtype=np.float32)
    b, c, h, w = x_full.shape
    assert b == N_CORES, f"expected batch {N_CORES}, got {b}"
    n_patches = c * h * w // S
    nc = build_kernel(n_patches)
    in_maps = make_in_maps(x_full, kmat)
    res = run_bass_kernel_spmd(nc, in_maps, core_ids=list(range(N_CORES)))
    out = np.stack(
        [
            unpack_output(res.results[i]["y"], n_patches).reshape(c, h, w)
            for i in range(b)
        ],
        axis=0,
    )
    return out


# revision 4
# speedup vs baseline: 2.1653x; 1.2898x over previous
"""Trainium2 Bass kernel for batched 64-point DCT (flattened-patch GEMM).

Reference computation: out = x.reshape(b, -1, 64) @ K, reshaped back.
Pure data parallel over 8 NeuronCores: core i handles batch i as a
[49152, 64] x [64, 64] GEMM. The kernel is HBM-bound, so the whole game
is minimizing bytes on the wire and keeping all DMA paths busy:

* Input travels as fp8 e3m4 (1 byte): host encodes with round-to-nearest
  via ml_dtypes; the device upcasts losslessly, so the quantization is
  fully host-controlled. Measured end-to-end rel err vs the fp32
  reference is 1.3e-2 against a 2e-2 gate (output fp16 adds ~5e-4).
* Output travels as fp16 (2 bytes); host upcasts to fp32.
* Device layout for BOTH tensors is [128, n_pairs]: partition
  r = z*64 + s (patch parity, coefficient), free dim = pair p
  (patch = 2p + z):  xth[z*64+s, p] = x[2p+z, s].
* Stationary operand = blockdiag(K, K) fp16, loaded once; each matmul
  streams 512 pair-columns into one half of a 2-bank PSUM tile:
      po[z*64+f, q] = sum_s K[s, f] * x[2q+z, s]
  so the output is produced directly in the input's (transposed) layout
  -- no on-chip transpose; the host un-transposes while upcasting.
* A single DMA queue on trn2 sustains only ~190 GB/s for 1 MB transfers
  (~2.3 us dead time between queued DMAs for the completion receipt),
  so loads AND stores are round-robined over all three DMA issuers
  (Sync HWDGE, Scalar HWDGE, GpSimd SWDGE) to reach the ~358 GB/s
  per-core HBM limit. Loads are emitted 3 tiles ahead of compute.
"""

import numpy as np
import ml_dtypes

import concourse.mybir as mybir
from concourse import bacc
from concourse.bass_utils import run_bass_kernel_spmd
from concourse.tile import TileContext

P = 128    # SBUF partitions
S = 64     # DCT size (contraction dim)
MM = 512   # moving columns per matmul (one PSUM bank of fp32)
N_CORES = 8
PAIRS_PER_TILE = 4096
MATMULS_PER_TILE = PAIRS_PER_TILE // MM   # 8
LOOKAHEAD = 3

# 'fp8_mixed':   x in fp8e3 DRAM+SBUF, matmul(lhsT=fp16, rhs=fp8e3)
# 'fp8_castdma': x in fp8e3 DRAM, gpsimd cast-DMA to fp16 SBUF
# 'fp16':        x in fp16 end-to-end
INPUT_MODE = "fp8_mixed"

IN_DT = mybir.dt.float8e3 if INPUT_MODE.startswith("fp8") else mybir.dt.float16
IN_NPDT = ml_dtypes.float8_e3m4 if INPUT_MODE.startswith("fp8") else np.float16
OUT_DT = mybir.dt.float16


def build_kernel(n_patches: int):
    assert n_patches % (2 * PAIRS_PER_TILE) == 0
    n_pairs = n_patches // 2
    n_tiles = n_pairs // PAIRS_PER_TILE
    nc = bacc.Bacc(
        "TRN2",
        target_bir_lowering=False,
        debug=False,
        enable_asserts=False,
        num_devices=N_CORES,
    )
    x = nc.dram_tensor("x", [P, n_pairs], IN_DT, kind="ExternalInput")
    k = nc.dram_tensor("k", [P, P], mybir.dt.float16, kind="ExternalInput")
    y = nc.dram_tensor("y", [P, n_pairs], OUT_DT, kind="ExternalOutput")

    xv = x.ap().rearrange("r (t n) -> t r n", n=PAIRS_PER_TILE)
    yv = y.ap().rearrange("r (t n) -> t r n", n=PAIRS_PER_TILE)

    with TileContext(nc) as tc:
        with (
            tc.tile_pool(name="consts", bufs=1) as consts,
            tc.tile_pool(name="xin", bufs=LOOKAHEAD + 2) as x_pool,
            tc.tile_pool(name="outsb", bufs=3) as out_pool,
            tc.tile_pool(name="pout", bufs=4, space="PSUM") as pout_pool,
        ):
            kblk = consts.tile([P, P], mybir.dt.float16)
            rings = [nc.sync, nc.scalar, nc.gpsimd]

            def load_ring(t):
                return rings[t % 3]

            def store_ring(t):
                return rings[(t + 1) % 3]

            x_tiles = {}

            def emit_load(t):
                buf = x_pool.tile(
                    [P, PAIRS_PER_TILE], IN_DT, tag="x_tile",
                    name=f"x{t}",
                )
                load_ring(t).dma_start(out=buf[:], in_=xv[t])
                x_tiles[t] = buf

            # kblk rides scalar ahead of L1; loads prefetch 3 deep
            emit_load(0)
            nc.scalar.dma_start(out=kblk[:], in_=k.ap())
            for t in range(1, min(LOOKAHEAD, n_tiles)):
                emit_load(t)

            for ti in range(n_tiles):
                if ti + LOOKAHEAD < n_tiles:
                    emit_load(ti + LOOKAHEAD)
                x_tile = x_tiles.pop(ti)
                if INPUT_MODE == "fp8_castdma":
                    x_use = x_pool.tile(
                        [P, PAIRS_PER_TILE], mybir.dt.float16,
                        tag="x_cast", name=f"xc{ti}",
                    )
                    nc.gpsimd.dma_start(out=x_use[:], in_=x_tile[:])
                else:
                    x_use = x_tile
                out_sb = out_pool.tile([P, PAIRS_PER_TILE], OUT_DT)
                for g in range(MATMULS_PER_TILE // 2):
                    po = pout_pool.tile([P, 2 * MM], mybir.dt.float32)
                    for half in range(2):
                        c0 = (2 * g + half) * MM
                        nc.tensor.matmul(
                            po[:, half * MM : (half + 1) * MM],
                            lhsT=kblk[:],
                            rhs=x_use[:, c0 : c0 + MM],
                            start=True,
                            stop=True,
                        )
                    dst = out_sb[:, 2 * g * MM : 2 * (g + 1) * MM]
                    if g % 2 == 0:
                        nc.vector.tensor_copy(dst, po[:])
                    else:
                        nc.scalar.copy(dst, po[:])
                store_ring(ti).dma_start(out=yv[ti], in_=out_sb[:])
    nc.compile()
    return nc


def pack_input(x_core: np.ndarray) -> np.ndarray:
    """[n_patches, 64] fp32 -> [128, n_pairs] device layout."""
    x3 = x_core.reshape(-1, 2, S)                     # [pair, z, s]
    return np.ascontiguousarray(
        x3.transpose(1, 2, 0).reshape(P, -1).astype(IN_NPDT)
    )


def unpack_output(y_dev: np.ndarray, n_patches: int) -> np.ndarray:
    """[128, n_pairs] fp16 device layout -> [n_patches, 64] fp32."""
    y3 = np.asarray(y_dev, dtype=np.float32).reshape(2, S, n_patches // 2)
    return y3.transpose(2, 0, 1).reshape(n_patches, S)


def make_in_maps(x_full: np.ndarray, kmat: np.ndarray) -> list[dict]:
    b = x_full.shape[0]
    n_patches = x_full[0].size // S
    kblk_host = np.zeros((P, P), dtype=np.float16)
    kblk_host[:S, :S] = kmat.astype(np.float16)
    kblk_host[S:, S:] = kmat.astype(np.float16)
    return [
        {"x": pack_input(x_full[i].reshape(n_patches, S)), "k": kblk_host}
        for i in range(b)
    ]


def kernel(inputs, kernel):
    x_full = np.asarray(inputs, dtype=np.float32)
    kmat = np.asarray(kernel, dtype=np.float32)
    b, c, h, w = x_full.shape
    assert b == N_CORES, f"expected batch {N_CORES}, got {b}"
    n_patches = c * h * w // S
    nc = build_kernel(n_patches)
    in_maps = make_in_maps(x_full, kmat)
    res = run_bass_kernel_spmd(nc, in_maps, core_ids=list(range(N_CORES)))
    out = np.stack(
        [
            unpack_output(res.results[i]["y"], n_patches).reshape(c, h, w)
            for i in range(b)
        ],
        axis=0,
    )
    return out
